# revision 3
# baseline (speedup 1.0000x reference)
"""Trainium2 Bass kernel for NSPhysicsLoss.

L = mean((u_pred-u_true)^2) + 0.001 * mean(res_u^2 + res_v^2)
res_c = (c - c_prev)/DT + u*c_x + v*c_y - NU*(grad(c_x,-1) + grad(c_y,-2))
with jnp.gradient semantics (central diff /2 interior, one-sided edges).

Reformulation with D = 2*gradient (raw central diff, edges pre-scaled x2):
res_c = SCALE_UT*(c - c_prev) + 0.5*u.*Dx(c) + 0.5*v.*Dy(c)
        + NSCALE*(DxDx(c) + DyDy(c)),
SCALE_UT = 1/f32(0.01), NSCALE = -f32(0.001)/4.

Per core (batch-parallel, 4 samples each): image [512,512] laid out as
[p=128, q=4, w=512] (h = q*128 + p). Dx along free dim on DVE/GPSIMD;
Dy via block-tridiagonal matmuls on PE (operator matrix embedded as
inline consts). Residual assembled entirely in PSUM by PE using scaled
identity matmuls; ACT squares with accum_out produce per-partition
partial sums; host does the final f64 reduction.
"""

import sys

import numpy as np

if "/opt/trn_rl_repo" not in sys.path:
    sys.path.insert(0, "/opt/trn_rl_repo")

from concourse import bacc, bass, mybir, tile
from concourse.bass_utils import run_bass_kernel_spmd

B, C, H, W = 32, 2, 512, 512
NCORES = 8
BS = B // NCORES  # samples per core
P = 128
Q = H // P  # 4 row blocks

DT_F = np.float32(0.01)
NU_F = np.float32(0.001)
LAM_F = np.float32(0.001)
SCALE_UT = float(np.float32(1.0) / DT_F)
NSCALE = float(-NU_F / np.float32(4.0))

AluOp = mybir.AluOpType
ActFn = mybir.ActivationFunctionType
F32 = mybir.dt.float32

# (qi, qo) pairs of the block-tridiagonal derivative operator
PAIRS = [(qi, qo) for qo in range(Q) for qi in range(Q) if abs(qi - qo) <= 1]
IDX = {pr: i for i, pr in enumerate(PAIRS)}

ID_I, ID_UT_P, ID_UT_M, ID_NS = 0, 1, 2, 3


def _build_consts():
    # D = 2*jnp.gradient matrix along one axis: interior f[i+1]-f[i-1],
    # edges 2*(f[1]-f[0]) / 2*(f[N-1]-f[N-2])
    A = np.zeros((H, H), dtype=np.float32)
    for i in range(1, H - 1):
        A[i, i - 1] = -1.0
        A[i, i + 1] = 1.0
    A[0, 0], A[0, 1] = -2.0, 2.0
    A[H - 1, H - 2], A[H - 1, H - 1] = -2.0, 2.0
    AT = A.T
    g = np.stack(
        [AT[qi * P:(qi + 1) * P, qo * P:(qo + 1) * P] for (qi, qo) in PAIRS],
        axis=1,
    )  # [128, 10, 128]
    gs = (np.float32(NSCALE) * g).astype(np.float32)
    eye = np.eye(P, dtype=np.float32)
    ids = np.stack(
        [
            eye,
            np.float32(SCALE_UT) * eye,
            np.float32(-SCALE_UT) * eye,
            np.float32(NSCALE) * eye,
        ],
        axis=1,
    )  # [128, 4, 128]
    return np.ascontiguousarray(g), np.ascontiguousarray(gs), np.ascontiguousarray(ids)


def build_program():
    g_np, gs_np, ids_np = _build_consts()
    nc = bacc.Bacc("TRN2", target_bir_lowering=False)

    up = nc.dram_tensor("u_pred", [BS, C, H, W], F32, kind="ExternalInput")
    ut = nc.dram_tensor("u_true", [BS, C, H, W], F32, kind="ExternalInput")
    uv = nc.dram_tensor("u_prev", [BS, C, H, W], F32, kind="ExternalInput")
    out_d = nc.dram_tensor("out", [P, 2], F32, kind="ExternalOutput")

    g_d = nc.inline_tensor(g_np, "g_const")
    gs_d = nc.inline_tensor(gs_np, "gs_const")
    id_d = nc.inline_tensor(ids_np, "id_const")

    with tile.TileContext(nc) as tc:
        with (
            tc.tile_pool(name="cpool", bufs=1) as cpool,
            tc.tile_pool(name="ppool", bufs=2) as ppool,
            tc.tile_pool(name="tpool", bufs=2) as tpool,
            tc.tile_pool(name="vpool", bufs=2) as vpool,
            tc.tile_pool(name="wpool", bufs=2) as wpool,
            tc.tile_pool(name="qpool", bufs=1, space=bass.MemorySpace.PSUM) as qpool,
        ):
            g_sb = cpool.tile([P, len(PAIRS), P], F32, tag="g_sb")
            gs_sb = cpool.tile([P, len(PAIRS), P], F32, tag="gs_sb")
            id_sb = cpool.tile([P, 4, P], F32, tag="id_sb")
            data_acc = cpool.tile([P, BS * C * Q], F32, tag="data_acc")
            phys_acc = cpool.tile([P, BS * C * Q], F32, tag="phys_acc")
            out_sb = cpool.tile([P, 2], F32, tag="out_sb")
            psum_sq = qpool.tile([P, W], F32, tag="psum_sq", bufs=1)

            nc.sync.dma_start(g_sb[:], g_d[:])
            nc.sync.dma_start(gs_sb[:], gs_d[:])
            nc.sync.dma_start(id_sb[:], id_d[:])

            def d_edges(out_t, src):
                # out[:, :, 0] = 2*(src[:,:,1]-src[:,:,0]); same at W-1
                o = out_t[:, :, 0:W:W - 1]
                nc.vector.tensor_sub(o, src[:, :, 1:W:W - 2], src[:, :, 0:W:W - 2])
                nc.vector.tensor_scalar_mul(o, o, 2.0)

            def d_interior(eng, out_t, src):
                eng.tensor_sub(
                    out_t[:, :, 1:W - 1], src[:, :, 2:W], src[:, :, 0:W - 2]
                )

            def emit_dy_mms(ch, qo):
                ps = qpool.tile([P, W], F32, tag="psum_dy", bufs=4, name="psum_dy")
                qis = [qi for qi in (qo - 1, qo, qo + 1) if 0 <= qi < Q]
                for j, qi in enumerate(qis):
                    nc.tensor.matmul(
                        ps[:], g_sb[:, IDX[(qi, qo)], :], ch[:, qi, :],
                        start=(j == 0), stop=(j == len(qis) - 1),
                    )
                return ps

            state = {}

            def emit_A(s, c, pred_t, true_t):
                ch = pred_t[:, c]
                u_ap = pred_t[:, 0]
                v_ap = pred_t[:, 1]
                k = s * C + c

                dx = wpool.tile([P, Q, W], F32, tag="dx", name="dx")
                dxx = wpool.tile([P, Q, W], F32, tag="dxx", name="dxx")
                dy_sb = wpool.tile([P, Q, W], F32, tag="dy_sb", name="dy_sb")
                p1t = wpool.tile([P, Q, W], F32, tag="p1t", name="p1t")
                p2t = wpool.tile([P, Q, W], F32, tag="p2t", name="p2t")
                tt = wpool.tile([P, Q, W], F32, tag="tt", name="tt")

                # gpsimd: data diff (c0,c1) and dx interior for c0
                if c == 0:
                    d_interior(nc.gpsimd, dx, ch)
                    nc.gpsimd.tensor_sub(tt[:], ch[:], true_t[:])
                else:
                    nc.gpsimd.tensor_sub(tt[:], ch[:], true_t[:])
                    d_interior(nc.vector, dx, ch)
                d_edges(dx, ch)

                # PE: Dy matmuls; ACT: psum->sbuf copies
                for qo in range(Q):
                    ps = emit_dy_mms(ch, qo)
                    nc.scalar.copy(dy_sb[:, qo, :], ps[:])

                # ACT: data-loss squares (after gpsimd diff)
                for qo in range(Q):
                    nc.scalar.activation(
                        psum_sq[:], tt[:, qo, :], ActFn.Square,
                        accum_out=data_acc[:, k * Q + qo: k * Q + qo + 1],
                    )

                # DVE: second derivative + advection products
                d_interior(nc.vector, dxx, dx)
                d_edges(dxx, dx)
                nc.vector.scalar_tensor_tensor(
                    p1t[:], u_ap[:], 0.5, dx[:], AluOp.mult, AluOp.mult
                )
                nc.vector.scalar_tensor_tensor(
                    p2t[:], v_ap[:], 0.5, dy_sb[:], AluOp.mult, AluOp.mult
                )

                state[(s, c)] = dict(
                    pred=pred_t, dxx=dxx, dy=dy_sb, p1=p1t, p2=p2t
                )

            def emit_B(s, c, prev_t):
                st = state.pop((s, c))
                ch = st["pred"][:, c]
                k = s * C + c
                for qo in range(Q):
                    pr = qpool.tile([P, W], F32, tag="psum_res", bufs=3, name="psum_res")
                    nc.tensor.matmul(
                        pr[:], id_sb[:, ID_UT_P, :], ch[:, qo, :],
                        start=True, stop=False,
                    )
                    nc.tensor.matmul(
                        pr[:], id_sb[:, ID_UT_M, :], prev_t[:, qo, :],
                        start=False, stop=False,
                    )
                    nc.tensor.matmul(
                        pr[:], id_sb[:, ID_I, :], st["p1"][:, qo, :],
                        start=False, stop=False,
                    )
                    nc.tensor.matmul(
                        pr[:], id_sb[:, ID_I, :], st["p2"][:, qo, :],
                        start=False, stop=False,
                    )
                    nc.tensor.matmul(
                        pr[:], id_sb[:, ID_NS, :], st["dxx"][:, qo, :],
                        start=False, stop=False,
                    )
                    qis = [qi for qi in (qo - 1, qo, qo + 1) if 0 <= qi < Q]
                    for j, qi in enumerate(qis):
                        nc.tensor.matmul(
                            pr[:], gs_sb[:, IDX[(qi, qo)], :], st["dy"][:, qi, :],
                            start=False, stop=(j == len(qis) - 1),
                        )
                    nc.scalar.activation(
                        psum_sq[:], pr[:], ActFn.Square,
                        accum_out=phys_acc[:, k * Q + qo: k * Q + qo + 1],
                    )

            for s in range(BS):
                pred_t = ppool.tile([P, C, Q, W], F32, tag="pred_t", name="pred_t")
                nc.sync.dma_start(
                    pred_t[:], up[s].rearrange("c (q p) w -> p c q w", p=P)
                )
                true_ts, prev_ts = [], []
                for c in range(C):
                    tr = tpool.tile([P, Q, W], F32, tag="true_t", name="true_t")
                    nc.sync.dma_start(
                        tr[:], ut[s, c].rearrange("(q p) w -> p q w", p=P)
                    )
                    true_ts.append(tr)
                    pv = vpool.tile([P, Q, W], F32, tag="prev_t", name="prev_t")
                    nc.sync.dma_start(
                        pv[:], uv[s, c].rearrange("(q p) w -> p q w", p=P)
                    )
                    prev_ts.append(pv)

                emit_A(s, 0, pred_t, true_ts[0])
                if s > 0:
                    emit_B(s - 1, 1, state_prev_prev)
                emit_A(s, 1, pred_t, true_ts[1])
                emit_B(s, 0, prev_ts[0])
                state_prev_prev = prev_ts[1]

            emit_B(BS - 1, 1, state_prev_prev)

            nc.vector.tensor_reduce(
                out_sb[:, 0:1], data_acc[:], axis=mybir.AxisListType.X, op=AluOp.add
            )
            nc.vector.tensor_reduce(
                out_sb[:, 1:2], phys_acc[:], axis=mybir.AxisListType.X, op=AluOp.add
            )
            nc.sync.dma_start(out_d[:], out_sb[:])

    nc.compile()
    return nc


_CACHE = {}
LAST_RESULT = None


def _get_program():
    if "nc" not in _CACHE:
        _CACHE["nc"] = build_program()
    return _CACHE["nc"]


def kernel(u_pred, u_true, u_prev):
    global LAST_RESULT
    nc = _get_program()
    in_maps = []
    for i in range(NCORES):
        sl = slice(i * BS, (i + 1) * BS)
        in_maps.append(
            {
                "u_pred": np.ascontiguousarray(u_pred[sl], dtype=np.float32),
                "u_true": np.ascontiguousarray(u_true[sl], dtype=np.float32),
                "u_prev": np.ascontiguousarray(u_prev[sl], dtype=np.float32),
            }
        )
    LAST_RESULT = run_bass_kernel_spmd(nc, in_maps, list(range(NCORES)))
    s_data = 0.0
    s_phys = 0.0
    for r in LAST_RESULT.results:
        o = np.asarray(r["out"], dtype=np.float64)
        s_data += float(o[:, 0].sum())
        s_phys += float(o[:, 1].sum())
    val = s_data / float(B * C * H * W) + float(LAM_F) * (s_phys / float(B * H * W))
    return np.asarray(val, dtype=np.float32)
